# revision 1
# baseline (speedup 1.0000x reference)
"""3-layer GCN (GCNConv x3) distributed over 8 NeuronCores.

Algorithm
---------
reference:  h1 = relu(S @ (x W0) + b0); h2 = relu(S @ (h1 W1) + b1);
            y  = S @ (h2 W2) + b2,  where S = norm-weighted adjacency plus
            self-loop diagonal (GCN symmetric normalization).
Because S @ (h W) == (S @ h) W, every layer is: aggregate (sparse) then a
tiny dense matmul.

Distribution: nodes are dealt round-robin by descending (in-degree+1) rank
to 8 cores; each core owns N/8 destination nodes in blocks of 128.  Edges
(plus one self-edge per node) are owned by the destination core and packed
into 128-edge tiles per (block, src-window).  The gather of h[src] rows uses
GPSIMD dma_gather (int16 indices => four <=32768-row windows of the node
table).  Per tile, a [128,128] selection matrix Eq[p,d] = (dstl[p]==d) is
built on DVE (one bf16 tensor_scalar is_equal against a constant iota), and
PE accumulates aggT[f,d] += sum_p msgs[p,f]*Eq[p,d] in PSUM over all of a
block's tiles.  msgs = gathered * coef is one batched DVE multiply per
gather chunk (coef = full GCN edge norm; pads have coef=0 and dstl=999).

Per layer: gather chunks (4 windows x blocks) -> scale -> Eq+matmul ->
per-block dense matmul W + bias/relu on ACT -> PE transpose back -> DMA to
local h -> AllGather into the next layer's shared table.
"""

import sys

sys.path.insert(0, "/opt/trn_rl_repo")

import numpy as np

from concourse import bass, bacc, mybir, tile
from concourse import bass_utils

P = 128
WIN = 32768  # int16 index window


def preprocess(x, edge_src, edge_dst, edge_weights, n_cores=8):
    N, F = x.shape
    E = edge_src.shape[0]
    assert N % n_cores == 0
    C = n_cores

    w64 = edge_weights.astype(np.float64)
    deg = np.bincount(edge_dst, weights=w64, minlength=N) + 1.0
    dinv = 1.0 / np.sqrt(deg)
    norm = (dinv[edge_src] * w64 * dinv[edge_dst]).astype(np.float32)
    self_coef = (dinv * dinv).astype(np.float32)

    indeg = np.bincount(edge_dst, minlength=N)
    rounds = indeg + 1

    # deal nodes by descending degree: rank r -> core r%C, pos r//C
    order = np.argsort(-rounds, kind="stable")
    core_of = np.empty(N, np.int64)
    pos_of = np.empty(N, np.int64)
    r = np.arange(N)
    core_of[order] = r % C
    pos_of[order] = r // C

    npc = N // C
    B = (npc + P - 1) // P
    rows_pc = B * P
    rows_total = C * rows_pc
    pid = core_of * rows_pc + pos_of
    blk_of = pos_of // P
    K = (rows_total + WIN - 1) // WIN  # src windows

    # edge stream entries: E real edges + N self edges, keyed by
    # (core, window, block); value: (lidx int16, dstl, coef)
    e_dst = np.concatenate([edge_dst, np.arange(N)])
    e_src = np.concatenate([edge_src, np.arange(N)])
    e_coef = np.concatenate([norm, self_coef])
    e_core = core_of[e_dst]
    e_blk = blk_of[e_dst]
    e_pid_src = pid[e_src]
    e_win = e_pid_src // WIN
    e_lidx = (e_pid_src % WIN).astype(np.int32)
    e_dstl = (pos_of[e_dst] % P).astype(np.int32)

    # group by (core, window, block); order within group arbitrary
    key = (e_core * K + e_win) * B + e_blk
    sort_e = np.argsort(key, kind="stable")
    key_s = key[sort_e]
    n_groups = C * K * B
    counts = np.bincount(key_s, minlength=n_groups).reshape(C, K, B)

    # tiles per (window, block): max over cores (same program on all cores)
    Tkb = np.maximum.reduce(-(-counts // P), axis=0)  # [K, B]
    # per-window column offsets of each block segment
    wloc_off = np.zeros((K, B + 1), np.int64)
    wloc_off[:, 1:] = np.cumsum(Tkb, axis=1)
    TOTk = wloc_off[:, -1].copy()          # cols per window
    win_base = np.zeros(K + 1, np.int64)
    win_base[1:] = np.cumsum(TOTk)
    TOT = int(win_base[-1])                # global cols

    # position of each edge within its (c,k,b) group
    first = np.zeros(n_groups + 1, np.int64)
    first[1:] = np.cumsum(counts.reshape(-1))
    jpos = np.arange(E + N) - first[key_s]

    col_w = wloc_off[e_win[sort_e], e_blk[sort_e]] + jpos // P  # window-local col
    col_g = win_base[e_win[sort_e]] + col_w                     # global col
    slot = jpos % P
    ecore = e_core[sort_e]

    # per-window int16 index streams, [C][K] -> flat [TOTk*128] then wrapped
    gidx_flat = [np.zeros((C, max(1, int(TOTk[k])) * P), np.int16) for k in range(K)]
    coef = np.zeros((C, P, TOT), np.float32)
    dstl = np.full((C, P, TOT), 999.0, np.float32)

    ew = e_win[sort_e]
    for k in range(K):
        m = ew == k
        gidx_flat[k][ecore[m], col_w[m] * P + slot[m]] = e_lidx[sort_e][m].astype(
            np.int16
        )
    coef[ecore, slot, col_g] = e_coef[sort_e]
    dstl[ecore, slot, col_g] = e_dstl[sort_e]

    # wrap in 16 partitions + replicate x8 across partition groups
    gidx16 = []
    for k in range(K):
        nidx = gidx_flat[k].shape[1]
        w = gidx_flat[k].reshape(C, nidx // 16, 16).transpose(0, 2, 1)  # [C,16,n/16]
        gidx16.append(np.tile(w, (1, 8, 1)).astype(np.int16))           # [C,128,n/16]

    xt = np.zeros((rows_total, F), np.float32)
    xt[pid] = np.asarray(x, np.float32)

    return dict(
        C=C, N=N, F=F, B=B, K=K,
        Tkb=Tkb, wloc_off=wloc_off, TOTk=[int(t) for t in TOTk],
        win_base=[int(w) for w in win_base], TOT=TOT,
        rows_pc=rows_pc, rows_total=rows_total,
        core_of=core_of, pos_of=pos_of,
        xt=xt, gidx16=gidx16, coef=coef, dstl=dstl,
    )


def build_nc(meta, blocks_per_chunk=4, skip_collective=False, scratch=16384, n_queues=1):
    C = meta["C"]; F = meta["F"]; B = meta["B"]; K = meta["K"]
    Tkb = meta["Tkb"]; wloc_off = meta["wloc_off"]
    TOTk = meta["TOTk"]; win_base = meta["win_base"]; TOT = meta["TOT"]
    rows_pc = meta["rows_pc"]; rows_total = meta["rows_total"]
    dt = mybir.dt
    f32 = dt.float32
    bf16 = dt.float16  # 16-bit compute dtype (fp16: exact 0/1, finer mantissa)

    # chunks of whole blocks; per window-call max cols for pool sizing
    chunks = [(b, min(b + blocks_per_chunk, B)) for b in range(0, B, blocks_per_chunk)]
    CHMAX = max(
        int(wloc_off[k][b1] - wloc_off[k][b0])
        for (b0, b1) in chunks for k in range(K)
    )

    nc = bacc.Bacc("TRN2", target_bir_lowering=False, debug=False, num_devices=C,
                   dynamic_dma_scratch_size=scratch, num_swdge_queues=n_queues)

    xt = nc.dram_tensor("xt", [rows_total, F], f32, kind="ExternalInput").ap()
    gixd = [
        nc.dram_tensor(f"gix{k}", [P, max(1, TOTk[k]) * 8], dt.int16,
                       kind="ExternalInput").ap()
        for k in range(K)
    ]
    coef_d = nc.dram_tensor("coef", [P, TOT], f32, kind="ExternalInput").ap()
    dstl_d = nc.dram_tensor("dstl", [P, TOT], f32, kind="ExternalInput").ap()
    iota_d = nc.dram_tensor("iota", [P, P], bf16, kind="ExternalInput").ap()  # fp16
    ident_d = nc.dram_tensor("ident", [F, F], f32, kind="ExternalInput").ap()
    w_d = [
        nc.dram_tensor(f"w{i}", [F, F if i < 2 else 1], f32, kind="ExternalInput").ap()
        for i in range(3)
    ]
    b_d = [
        nc.dram_tensor(f"b{i}", [F, 1], f32, kind="ExternalInput").ap()
        for i in range(2)
    ]
    y_d = nc.dram_tensor("y", [1, rows_pc], f32, kind="ExternalOutput").ap()

    hloc = [nc.dram_tensor(f"hloc{i}", [rows_pc, F], f32) for i in range(2)]
    htab = [
        nc.dram_tensor(f"htab{i}", [rows_total, F], f32, addr_space="Shared")
        for i in range(2)
    ]

    with tile.TileContext(nc) as tc:
        with (
            tc.tile_pool(name="const", bufs=1) as cpool,
            tc.tile_pool(name="gather", bufs=3) as gpool,
            tc.tile_pool(name="msgs", bufs=3) as mpool,
            tc.tile_pool(name="eqp", bufs=8) as epool,
            tc.tile_pool(name="aggs", bufs=3) as apool,
            tc.tile_pool(name="hout", bufs=3) as hpool,
            tc.tile_pool(name="psum_agg", bufs=4, space="PSUM") as ps_agg,
            tc.tile_pool(name="psum_dense", bufs=2, space="PSUM") as ps_dense,
            tc.tile_pool(name="psum_tr", bufs=2, space="PSUM") as ps_tr,
        ):
            gix_sb = [
                cpool.tile([P, max(1, TOTk[k]) * 8], dt.int16, tag=f"gix{k}",
                           name=f"gix{k}sb")
                for k in range(K)
            ]
            coef_sb = cpool.tile([P, TOT], f32, tag="coef")
            dstl_sb = cpool.tile([P, TOT], f32, tag="dstl")
            iota_sb = cpool.tile([P, P], bf16, tag="iota")
            ident_sb = cpool.tile([F, F], f32, tag="ident")
            w_sb = [cpool.tile([F, F if i < 2 else 1], f32, tag=f"w{i}",
                               name=f"w{i}sb") for i in range(3)]
            b_sb = [cpool.tile([F, 1], f32, tag=f"b{i}", name=f"b{i}sb")
                    for i in range(2)]
            y_sb = cpool.tile([1, rows_pc], f32, tag="ysb")

            for k in range(K):
                nc.sync.dma_start(out=gix_sb[k][:, :], in_=gixd[k][:, :])
            nc.sync.dma_start(out=coef_sb[:, :], in_=coef_d[:, :])
            nc.sync.dma_start(out=dstl_sb[:, :], in_=dstl_d[:, :])
            nc.sync.dma_start(out=iota_sb[:, :], in_=iota_d[:, :])
            nc.sync.dma_start(out=ident_sb[:, :], in_=ident_d[:, :])
            for i in range(3):
                nc.sync.dma_start(out=w_sb[i][:, :], in_=w_d[i][:, :])
            for i in range(2):
                nc.sync.dma_start(out=b_sb[i][:, :], in_=b_d[i][:, :])
            call_no = 0
            for L in range(3):
                table = [xt, htab[0][:, :], htab[1][:, :]][L]
                for (b0, b1) in chunks:
                    aggs_ps = {}
                    started = {}
                    for b in range(b0, b1):
                        aggs_ps[b] = ps_agg.tile([F, P], f32, tag="agg",
                                                 name=f"agg{L}_{b}")
                        started[b] = False
                    for k in range(K):
                        c0 = int(wloc_off[k][b0]); c1 = int(wloc_off[k][b1])
                        cols = c1 - c0
                        if cols == 0:
                            continue
                        g = gpool.tile([P, CHMAX * F], f32, tag="g")
                        nc.gpsimd.dma_gather(
                            out_ap=g[:, : cols * F].rearrange(
                                "p (t f) -> p t f", f=F),
                            in_ap=table[k * WIN: min((k + 1) * WIN, rows_total), :],
                            idxs_ap=gix_sb[k][:, c0 * 8: c1 * 8],
                            num_idxs=cols * P,
                            num_idxs_reg=cols * P,
                            elem_size=F,
                            single_packet=False,
                            queue_num=call_no % n_queues,
                        )
                        call_no += 1
                        m = mpool.tile([P, CHMAX * F], bf16, tag="m")
                        gc0 = win_base[k] + c0
                        gc1 = win_base[k] + c1
                        nc.vector.tensor_tensor(
                            out=m[:, : cols * F].rearrange("p (t f) -> p t f", f=F),
                            in0=g[:, : cols * F].rearrange("p (t f) -> p t f", f=F),
                            in1=coef_sb[:, gc0:gc1].to_broadcast([P, cols, F]),
                            op=mybir.AluOpType.mult,
                        )
                        for b in range(b0, b1):
                            nt = int(Tkb[k][b])
                            base = int(wloc_off[k][b]) - c0
                            for t in range(nt):
                                gcol = win_base[k] + c0 + base + t
                                eq = epool.tile([P, P], bf16, tag="eq")
                                nc.vector.tensor_scalar(
                                    out=eq[:, :],
                                    in0=iota_sb[:, :],
                                    scalar1=dstl_sb[:, gcol:gcol + 1],
                                    scalar2=None,
                                    op0=mybir.AluOpType.is_equal,
                                )
                                last = (k == K - 1 or all(
                                    Tkb[kk][b] == 0 for kk in range(k + 1, K)
                                )) and t == nt - 1
                                nc.tensor.matmul(
                                    aggs_ps[b][:, :],
                                    lhsT=m[:, (base + t) * F: (base + t + 1) * F],
                                    rhs=eq[:, :],
                                    start=not started[b],
                                    stop=last,
                                )
                                started[b] = True
                    for b in range(b0, b1):
                        aggs = apool.tile([F, P], f32, tag="aggs")
                        nc.scalar.activation(
                            aggs[:, :], aggs_ps[b][:, :],
                            mybir.ActivationFunctionType.Copy,
                        )
                        if L < 2:
                            hp = ps_dense.tile([F, P], f32, tag="hp")
                            nc.tensor.matmul(
                                hp[:, :], lhsT=w_sb[L][:, :], rhs=aggs[:, :],
                                start=True, stop=True,
                            )
                            hT = apool.tile([F, P], f32, tag="hT")
                            nc.scalar.activation(
                                hT[:, :], hp[:, :],
                                mybir.ActivationFunctionType.Relu,
                                bias=b_sb[L][:, :],
                            )
                            tp = ps_tr.tile([P, F], f32, tag="tp")
                            nc.tensor.matmul(
                                tp[:, :], lhsT=hT[:, :], rhs=ident_sb[:, :],
                                is_transpose=True, start=True, stop=True,
                            )
                            hout = hpool.tile([P, F], f32, tag="hout")
                            nc.scalar.activation(
                                hout[:, :], tp[:, :],
                                mybir.ActivationFunctionType.Copy,
                            )
                            nc.sync.dma_start(
                                out=hloc[L][b * P: (b + 1) * P, :], in_=hout[:, :]
                            )
                        else:
                            yp = ps_dense.tile([1, P], f32, tag="hp", name="yp")
                            nc.tensor.matmul(
                                yp[:, :], lhsT=w_sb[2][:, :], rhs=aggs[:, :],
                                start=True, stop=True,
                            )
                            nc.scalar.activation(
                                y_sb[:, b * P: (b + 1) * P], yp[:, :],
                                mybir.ActivationFunctionType.Copy,
                            )
                if L < 2 and not skip_collective:
                    nc.gpsimd.collective_compute(
                        "AllGather",
                        mybir.AluOpType.bypass,
                        replica_groups=[list(range(C))],
                        ins=[hloc[L].ap().opt()],
                        outs=[htab[L].ap().opt()],
                    )
            nc.sync.dma_start(out=y_d[:, :], in_=y_sb[:, :])

    nc.compile()
    return nc


def make_in_maps(meta, W0, b0, W1, b1, W2):
    C = meta["C"]; F = meta["F"]; K = meta["K"]
    import ml_dtypes
    iota = np.tile(np.arange(P), (P, 1)).astype(np.float16)
    common = dict(
        xt=meta["xt"],
        coef=meta["coef"],  # per-core below
        iota=iota,
        ident=np.eye(F, dtype=np.float32),
        w0=np.asarray(W0, np.float32),
        w1=np.asarray(W1, np.float32),
        w2=np.asarray(W2, np.float32).reshape(F, 1),
        b0=np.asarray(b0, np.float32).reshape(F, 1),
        b1=np.asarray(b1, np.float32).reshape(F, 1),
    )
    in_maps = []
    for c in range(C):
        im = dict(common)
        im["coef"] = meta["coef"][c]
        im["dstl"] = meta["dstl"][c]
        for k in range(K):
            im[f"gix{k}"] = meta["gidx16"][k][c]
        in_maps.append(im)
    return in_maps


def assemble_output(meta, results, b2):
    C = meta["C"]
    rows_pc = meta["rows_pc"]
    ys = np.stack([np.asarray(results[c]["y"]).reshape(rows_pc) for c in range(C)])
    y = ys[meta["core_of"], meta["pos_of"]] + np.float32(np.asarray(b2).reshape(-1)[0])
    return y.astype(np.float32)


def kernel(x, edge_src, edge_dst, edge_weights, W0, b0, W1, b1, W2, b2,
           blocks_per_chunk=4, n_queues=4, trace=False):
    """Harness entry point: full inputs in, full [N] float32 output."""
    x = np.asarray(x)
    meta = preprocess(x, np.asarray(edge_src), np.asarray(edge_dst),
                      np.asarray(edge_weights))
    nc = build_nc(meta, blocks_per_chunk=blocks_per_chunk, n_queues=n_queues)
    in_maps = make_in_maps(meta, W0, b0, W1, b1, W2)
    last_err = None
    for attempt in range(3):
        try:
            res = bass_utils.run_bass_kernel_spmd(
                nc, in_maps, core_ids=list(range(meta["C"])), trace=trace
            )
            y = assemble_output(meta, res.results, b2)
            kernel.last_result = res
            return y
        except Exception as e:  # transient accelerator failures: retry
            last_err = e
    raise last_err



# revision 11
# speedup vs baseline: 7.7559x; 7.7559x over previous
"""3-layer GCN (GCNConv x3) distributed over 8 NeuronCores.

Algorithm
---------
reference:  h1 = relu(S @ (x W0) + b0); h2 = relu(S @ (h1 W1) + b1);
            y  = S @ (h2 W2) + b2,  where S = norm-weighted adjacency plus
            self-loop diagonal (GCN symmetric normalization).
Because S @ (h W) == (S @ h) W, every layer is: aggregate (sparse) then a
tiny dense matmul.

Distribution: nodes are dealt round-robin by descending (in-degree+1) rank
to 8 cores; each core owns N/8 destination nodes in blocks of 128.  Edges
(plus one self-edge per node) are owned by the destination core and packed
into 128-edge tiles per (block, src-window).  The gather of h[src] rows uses
GPSIMD dma_gather (int16 indices => four <=32768-row windows of the node
table).  Per tile, a [128,128] selection matrix Eq[p,d] = (dstl[p]==d) is
built on DVE (one bf16 tensor_scalar is_equal against a constant iota), and
PE accumulates aggT[f,d] += sum_p msgs[p,f]*Eq[p,d] in PSUM over all of a
block's tiles.  msgs = gathered * coef is one batched DVE multiply per
gather chunk (coef = full GCN edge norm; pads have coef=0 and dstl=999).

Per layer: gather chunks (4 windows x blocks) -> scale -> Eq+matmul ->
per-block dense matmul W + bias/relu on ACT -> PE transpose back -> DMA to
local h -> AllGather into the next layer's shared table.
"""

import sys

sys.path.insert(0, "/opt/trn_rl_repo")

import numpy as np

from concourse import bass, bacc, mybir, tile
from concourse import bass_utils

P = 128
WIN = 32768  # int16 index window


def preprocess(x, edge_src, edge_dst, edge_weights, n_cores=8):
    N, F = x.shape
    E = edge_src.shape[0]
    assert N % n_cores == 0
    C = n_cores

    w64 = edge_weights.astype(np.float64)
    deg = np.bincount(edge_dst, weights=w64, minlength=N) + 1.0
    dinv = 1.0 / np.sqrt(deg)
    norm = (dinv[edge_src] * w64 * dinv[edge_dst]).astype(np.float32)
    self_coef = (dinv * dinv).astype(np.float32)

    indeg = np.bincount(edge_dst, minlength=N)
    rounds = indeg + 1

    # deal nodes by descending degree: rank r -> core r%C, pos r//C
    order = np.argsort(-rounds, kind="stable")
    core_of = np.empty(N, np.int64)
    pos_of = np.empty(N, np.int64)
    r = np.arange(N)
    core_of[order] = r % C
    pos_of[order] = r // C

    npc = N // C
    B = (npc + P - 1) // P
    rows_pc = B * P
    rows_total = C * rows_pc
    pid = core_of * rows_pc + pos_of
    blk_of = pos_of // P
    K = (rows_total + WIN - 1) // WIN  # src windows

    # edge stream entries: E real edges + N self edges, keyed by
    # (core, window, block); value: (lidx int16, dstl, coef)
    e_dst = np.concatenate([edge_dst, np.arange(N)])
    e_src = np.concatenate([edge_src, np.arange(N)])
    e_coef = np.concatenate([norm, self_coef])
    e_core = core_of[e_dst]
    e_blk = blk_of[e_dst]
    e_pid_src = pid[e_src]
    e_win = e_pid_src // WIN
    e_lidx = (e_pid_src % WIN).astype(np.int32)
    e_dstl = (pos_of[e_dst] % P).astype(np.int32)

    # group by (core, window, block); order within group arbitrary
    key = (e_core * K + e_win) * B + e_blk
    sort_e = np.argsort(key, kind="stable")
    key_s = key[sort_e]
    n_groups = C * K * B
    counts = np.bincount(key_s, minlength=n_groups).reshape(C, K, B)

    # tiles per (window, block): max over cores (same program on all cores)
    Tkb = np.maximum.reduce(-(-counts // P), axis=0)  # [K, B]
    # per-window column offsets of each block segment
    wloc_off = np.zeros((K, B + 1), np.int64)
    wloc_off[:, 1:] = np.cumsum(Tkb, axis=1)
    TOTk = wloc_off[:, -1].copy()          # cols per window
    win_base = np.zeros(K + 1, np.int64)
    win_base[1:] = np.cumsum(TOTk)
    TOT = int(win_base[-1])                # global cols

    # position of each edge within its (c,k,b) group
    first = np.zeros(n_groups + 1, np.int64)
    first[1:] = np.cumsum(counts.reshape(-1))
    jpos = np.arange(E + N) - first[key_s]

    col_w = wloc_off[e_win[sort_e], e_blk[sort_e]] + jpos // P  # window-local col
    col_g = win_base[e_win[sort_e]] + col_w                     # global col
    slot = jpos % P
    ecore = e_core[sort_e]

    # per-window int16 index streams, [C][K] -> flat [TOTk*128] then wrapped
    gidx_flat = [np.zeros((C, max(1, int(TOTk[k])) * P), np.int16) for k in range(K)]
    coef = np.zeros((C, P, TOT), np.float32)
    dstl = np.full((C, P, TOT), 999.0, np.float32)

    ew = e_win[sort_e]
    for k in range(K):
        m = ew == k
        gidx_flat[k][ecore[m], col_w[m] * P + slot[m]] = e_lidx[sort_e][m].astype(
            np.int16
        )
    coef[ecore, slot, col_g] = e_coef[sort_e]
    dstl[ecore, slot, col_g] = e_dstl[sort_e]

    # wrap in 16 partitions + replicate x8 across partition groups
    gidx16 = []
    for k in range(K):
        nidx = gidx_flat[k].shape[1]
        w = gidx_flat[k].reshape(C, nidx // 16, 16).transpose(0, 2, 1)  # [C,16,n/16]
        gidx16.append(np.tile(w, (1, 8, 1)).astype(np.int16))           # [C,128,n/16]

    xt = np.zeros((rows_total, F), np.float32)
    xt[pid] = np.asarray(x, np.float32)

    return dict(
        C=C, N=N, F=F, B=B, K=K,
        Tkb=Tkb, wloc_off=wloc_off, TOTk=[int(t) for t in TOTk],
        win_base=[int(w) for w in win_base], TOT=TOT,
        rows_pc=rows_pc, rows_total=rows_total,
        core_of=core_of, pos_of=pos_of,
        xt=xt, gidx16=gidx16, coef=coef, dstl=dstl,
    )


def build_nc(meta, blocks_per_chunk=4, skip_collective=False, scratch=16384,
             n_queues=1, ablate=frozenset()):
    C = meta["C"]; F = meta["F"]; B = meta["B"]; K = meta["K"]
    Tkb = meta["Tkb"]; wloc_off = meta["wloc_off"]
    TOTk = meta["TOTk"]; win_base = meta["win_base"]; TOT = meta["TOT"]
    rows_pc = meta["rows_pc"]; rows_total = meta["rows_total"]
    dt = mybir.dt
    f32 = dt.float32
    bf16 = dt.float16  # 16-bit compute dtype (fp16: exact 0/1, finer mantissa)

    # chunks of whole blocks; per window-call max cols for pool sizing
    chunks = [(b, min(b + blocks_per_chunk, B)) for b in range(0, B, blocks_per_chunk)]
    CHMAX = max(
        int(wloc_off[k][b1] - wloc_off[k][b0])
        for (b0, b1) in chunks for k in range(K)
    )

    nc = bacc.Bacc("TRN2", target_bir_lowering=False, debug=False, num_devices=C,
                   dynamic_dma_scratch_size=scratch, num_swdge_queues=n_queues)

    xt = nc.dram_tensor("xt", [rows_total, F], f32, kind="ExternalInput").ap()
    gixd = [
        nc.dram_tensor(f"gix{k}", [P, max(1, TOTk[k]) * 8], dt.int16,
                       kind="ExternalInput").ap()
        for k in range(K)
    ]
    coef_d = nc.dram_tensor("coef", [P, TOT], f32, kind="ExternalInput").ap()
    dstl_d = nc.dram_tensor("dstl", [P, TOT], f32, kind="ExternalInput").ap()
    iota_d = nc.dram_tensor("iota", [P, P], bf16, kind="ExternalInput").ap()  # fp16
    ident_d = nc.dram_tensor("ident", [F, F], f32, kind="ExternalInput").ap()
    w_d = [
        nc.dram_tensor(f"w{i}", [F, F if i < 2 else 1], f32, kind="ExternalInput").ap()
        for i in range(3)
    ]
    b_d = [
        nc.dram_tensor(f"b{i}", [F, 1], f32, kind="ExternalInput").ap()
        for i in range(2)
    ]
    y_d = nc.dram_tensor("y", [1, rows_pc], f32, kind="ExternalOutput").ap()

    hloc = [nc.dram_tensor(f"hloc{i}", [rows_pc, F], f32) for i in range(2)]
    htab = [
        nc.dram_tensor(f"htab{i}", [rows_total, F], f32, addr_space="Shared")
        for i in range(2)
    ]

    with tile.TileContext(nc) as tc:
        with (
            tc.tile_pool(name="const", bufs=1) as cpool,
            tc.tile_pool(name="gather", bufs=3) as gpool,
            tc.tile_pool(name="msgs", bufs=3) as mpool,
            tc.tile_pool(name="eqp", bufs=8) as epool,
            tc.tile_pool(name="aggs", bufs=3) as apool,
            tc.tile_pool(name="hout", bufs=3) as hpool,
            tc.tile_pool(name="psum_agg", bufs=4, space="PSUM") as ps_agg,
            tc.tile_pool(name="psum_dense", bufs=2, space="PSUM") as ps_dense,
            tc.tile_pool(name="psum_tr", bufs=2, space="PSUM") as ps_tr,
        ):
            gix_sb = [
                cpool.tile([P, max(1, TOTk[k]) * 8], dt.int16, tag=f"gix{k}",
                           name=f"gix{k}sb")
                for k in range(K)
            ]
            coef_sb = cpool.tile([P, TOT], f32, tag="coef")
            dstl_sb = cpool.tile([P, TOT], f32, tag="dstl")
            iota_sb = cpool.tile([P, P], bf16, tag="iota")
            ident_sb = cpool.tile([F, F], f32, tag="ident")
            w_sb = [cpool.tile([F, F if i < 2 else 1], f32, tag=f"w{i}",
                               name=f"w{i}sb") for i in range(3)]
            b_sb = [cpool.tile([F, 1], f32, tag=f"b{i}", name=f"b{i}sb")
                    for i in range(2)]
            y_sb = cpool.tile([1, rows_pc], f32, tag="ysb")

            for k in range(K):
                nc.sync.dma_start(out=gix_sb[k][:, :], in_=gixd[k][:, :])
            nc.sync.dma_start(out=coef_sb[:, :], in_=coef_d[:, :])
            nc.sync.dma_start(out=dstl_sb[:, :], in_=dstl_d[:, :])
            nc.sync.dma_start(out=iota_sb[:, :], in_=iota_d[:, :])
            nc.sync.dma_start(out=ident_sb[:, :], in_=ident_d[:, :])
            for i in range(3):
                nc.sync.dma_start(out=w_sb[i][:, :], in_=w_d[i][:, :])
            for i in range(2):
                nc.sync.dma_start(out=b_sb[i][:, :], in_=b_d[i][:, :])
            call_no = 0
            for L in range(3):
                table = [xt, htab[0][:, :], htab[1][:, :]][L]
                for (b0, b1) in chunks:
                    aggs_ps = {}
                    started = {}
                    for b in range(b0, b1):
                        aggs_ps[b] = ps_agg.tile([F, P], f32, tag="agg",
                                                 name=f"agg{L}_{b}")
                        started[b] = False
                    for k in range(K):
                        c0 = int(wloc_off[k][b0]); c1 = int(wloc_off[k][b1])
                        cols = c1 - c0
                        if cols == 0:
                            continue
                        g = gpool.tile([P, CHMAX * F], f32, tag="g")
                        if "gather" in ablate:
                            # streaming stand-in: same bytes, contiguous
                            nc.sync.dma_start(
                                out=g[:, : cols * F].rearrange(
                                    "p (t f) -> p t f", f=F),
                                in_=table[k * WIN: k * WIN + cols * P, :]
                                .rearrange("(p t) f -> p t f", p=P),
                            )
                        else:
                            nc.gpsimd.dma_gather(
                                out_ap=g[:, : cols * F].rearrange(
                                    "p (t f) -> p t f", f=F),
                                in_ap=table[k * WIN: min((k + 1) * WIN, rows_total), :],
                                idxs_ap=gix_sb[k][:, c0 * 8: c1 * 8],
                                num_idxs=cols * P,
                                num_idxs_reg=cols * P,
                                elem_size=F,
                                single_packet=False,
                                queue_num=call_no % n_queues,
                            )
                        call_no += 1
                        m = mpool.tile([P, CHMAX * F], bf16, tag="m")
                        gc0 = win_base[k] + c0
                        gc1 = win_base[k] + c1
                        # f32 -> f16 convert on ACT; coef is folded into Eq
                        nc.scalar.activation(
                            m[:, : cols * F], g[:, : cols * F],
                            mybir.ActivationFunctionType.Copy,
                        )
                        for b in range(b0, b1):
                            nt = int(Tkb[k][b])
                            base = int(wloc_off[k][b]) - c0
                            for t in range(nt):
                                gcol = win_base[k] + c0 + base + t
                                if "eq" in ablate:
                                    eq = iota_sb
                                else:
                                    # eq[p,d] = (iota==dstl[p]) * coef[p]
                                    eq = epool.tile([P, P], bf16, tag="eq")
                                    nc.vector.tensor_scalar(
                                        out=eq[:, :],
                                        in0=iota_sb[:, :],
                                        scalar1=dstl_sb[:, gcol:gcol + 1],
                                        scalar2=coef_sb[:, gcol:gcol + 1],
                                        op0=mybir.AluOpType.is_equal,
                                        op1=mybir.AluOpType.mult,
                                    )
                                last = (k == K - 1 or all(
                                    Tkb[kk][b] == 0 for kk in range(k + 1, K)
                                )) and t == nt - 1
                                if "agg" not in ablate:
                                    nc.tensor.matmul(
                                        aggs_ps[b][:, :],
                                        lhsT=m[:, (base + t) * F: (base + t + 1) * F],
                                        rhs=eq[:, :],
                                        start=not started[b],
                                        stop=last,
                                    )
                                    started[b] = True
                                elif last:
                                    # single matmul per block keeps PSUM valid
                                    nc.tensor.matmul(
                                        aggs_ps[b][:, :],
                                        lhsT=m[:, (base + t) * F: (base + t + 1) * F],
                                        rhs=eq[:, :],
                                        start=True,
                                        stop=True,
                                    )
                    for b in range(b0, b1):
                        aggs = apool.tile([F, P], f32, tag="aggs")
                        nc.scalar.activation(
                            aggs[:, :], aggs_ps[b][:, :],
                            mybir.ActivationFunctionType.Copy,
                        )
                        if L < 2:
                            hp = ps_dense.tile([F, P], f32, tag="hp")
                            nc.tensor.matmul(
                                hp[:, :], lhsT=w_sb[L][:, :], rhs=aggs[:, :],
                                start=True, stop=True,
                            )
                            hT = apool.tile([F, P], f32, tag="hT")
                            nc.scalar.activation(
                                hT[:, :], hp[:, :],
                                mybir.ActivationFunctionType.Relu,
                                bias=b_sb[L][:, :],
                            )
                            tp = ps_tr.tile([P, F], f32, tag="tp")
                            nc.tensor.matmul(
                                tp[:, :], lhsT=hT[:, :], rhs=ident_sb[:, :],
                                is_transpose=True, start=True, stop=True,
                            )
                            hout = hpool.tile([P, F], f32, tag="hout")
                            nc.scalar.activation(
                                hout[:, :], tp[:, :],
                                mybir.ActivationFunctionType.Copy,
                            )
                            nc.sync.dma_start(
                                out=hloc[L][b * P: (b + 1) * P, :], in_=hout[:, :]
                            )
                        else:
                            yp = ps_dense.tile([1, P], f32, tag="hp", name="yp")
                            nc.tensor.matmul(
                                yp[:, :], lhsT=w_sb[2][:, :], rhs=aggs[:, :],
                                start=True, stop=True,
                            )
                            nc.scalar.activation(
                                y_sb[:, b * P: (b + 1) * P], yp[:, :],
                                mybir.ActivationFunctionType.Copy,
                            )
                if L < 2 and not skip_collective:
                    nc.gpsimd.collective_compute(
                        "AllGather",
                        mybir.AluOpType.bypass,
                        replica_groups=[list(range(C))],
                        ins=[hloc[L].ap().opt()],
                        outs=[htab[L].ap().opt()],
                    )
            nc.sync.dma_start(out=y_d[:, :], in_=y_sb[:, :])

    nc.compile()
    return nc


def make_in_maps(meta, W0, b0, W1, b1, W2):
    C = meta["C"]; F = meta["F"]; K = meta["K"]
    import ml_dtypes
    iota = np.tile(np.arange(P), (P, 1)).astype(np.float16)
    common = dict(
        xt=meta["xt"],
        coef=meta["coef"],  # per-core below
        iota=iota,
        ident=np.eye(F, dtype=np.float32),
        w0=np.asarray(W0, np.float32),
        w1=np.asarray(W1, np.float32),
        w2=np.asarray(W2, np.float32).reshape(F, 1),
        b0=np.asarray(b0, np.float32).reshape(F, 1),
        b1=np.asarray(b1, np.float32).reshape(F, 1),
    )
    in_maps = []
    for c in range(C):
        im = dict(common)
        im["coef"] = meta["coef"][c]
        im["dstl"] = meta["dstl"][c]
        for k in range(K):
            im[f"gix{k}"] = meta["gidx16"][k][c]
        in_maps.append(im)
    return in_maps


def assemble_output(meta, results, b2):
    C = meta["C"]
    rows_pc = meta["rows_pc"]
    ys = np.stack([np.asarray(results[c]["y"]).reshape(rows_pc) for c in range(C)])
    y = ys[meta["core_of"], meta["pos_of"]] + np.float32(np.asarray(b2).reshape(-1)[0])
    return y.astype(np.float32)


def kernel(x, edge_src, edge_dst, edge_weights, W0, b0, W1, b1, W2, b2,
           blocks_per_chunk=4, n_queues=4, scratch=65536, trace=False):
    """Harness entry point: full inputs in, full [N] float32 output."""
    x = np.asarray(x)
    meta = preprocess(x, np.asarray(edge_src), np.asarray(edge_dst),
                      np.asarray(edge_weights))
    nc = build_nc(meta, blocks_per_chunk=blocks_per_chunk, n_queues=n_queues,
                  scratch=scratch)
    in_maps = make_in_maps(meta, W0, b0, W1, b1, W2)
    last_err = None
    for attempt in range(3):
        try:
            res = bass_utils.run_bass_kernel_spmd(
                nc, in_maps, core_ids=list(range(meta["C"])), trace=trace
            )
            y = assemble_output(meta, res.results, b2)
            kernel.last_result = res
            return y
        except Exception as e:  # transient accelerator failures: retry
            last_err = e
    raise last_err

